# revision 14
# baseline (speedup 1.0000x reference)
"""Trainium2 8-core tensor-parallel attention kernel (Bass/Tile), v3.

Full inputs in, full output out. Sharding: tensor-parallel over heads
(4 heads per core), per-batch AllGather of attention outputs overlapped
with compute, each core computes a 512-wide output-column slice of the
o_proj; host concatenates.

v3 vs v2:
- Attention row-sums moved off the tensor engine: per-query key-sums are
  built on the vector engine (bf16 chunk collapses + f32 folds) and the
  128-partition collapse + broadcast is a single 512-col ones-matmul per
  query block (was: a full third matmul stream + 32-row broadcast).
- Exp batched: scores for up to 3 key blocks accumulate in a 3-bank PSUM
  chunk and drain through ONE 1536-col activation (amortizes the 352-cycle
  ACT instruction overhead ~3x).
- Causal mask via affine_select on the exp output, run on the otherwise-
  idle gpsimd engine: no psum mask adds, and it replaces (not multiplies)
  the untouched (garbage) psum region of column-trimmed diagonal blocks,
  so exp(garbage)=Inf/NaN cannot leak into the sums.
- PSUM pools are phase-siblings (p1P -> attnP -> opP), each phase gets
  all 8 banks; phase-1 projection groups double-buffer 4-deep.
- Softmax scale folded into exp; normalization multiply on vector.
"""
import sys

for _p in ("/opt/trn_rl_repo",):
    if _p not in sys.path:
        sys.path.insert(0, _p)

import numpy as np
import ml_dtypes

import concourse.bass as bass
import concourse.mybir as mybir
import concourse.tile as tile
from concourse import bacc
from concourse.bass_utils import run_bass_kernel_spmd

B, S, D, H = 2, 2048, 4096, 32
HD = D // H          # 128 head dim
T = B * S            # 4096 tokens
NC = 8               # cores
HL = H // NC         # 4 heads per core
DH = HL * HD         # 512 dims per core
SCALE = 1.0 / float(np.sqrt(HD))
BF16 = mybir.dt.bfloat16
F32 = mybir.dt.float32
bf16 = ml_dtypes.bfloat16

_CACHE = {}
LAST_RESULT = None

CH = 3  # key blocks per psum score chunk (3 banks)


def build():
    nc = bacc.Bacc("TRN2", target_bir_lowering=False, debug=False, num_devices=NC)

    xT = nc.dram_tensor("xT", [D, T], BF16, kind="ExternalInput").ap()
    wqT = nc.dram_tensor("wqT", [D, DH], BF16, kind="ExternalInput").ap()
    wkT = nc.dram_tensor("wkT", [D, DH], BF16, kind="ExternalInput").ap()
    wvT = nc.dram_tensor("wvT", [D, DH], BF16, kind="ExternalInput").ap()
    woT = nc.dram_tensor("woT", [D, DH], BF16, kind="ExternalInput").ap()
    cos2E = nc.dram_tensor("cos2E", [HD, T], BF16, kind="ExternalInput").ap()
    sin2E = nc.dram_tensor("sin2E", [HD, T], BF16, kind="ExternalInput").ap()
    ones_b = nc.dram_tensor("ones_b", [128, 128], BF16, kind="ExternalInput").ap()
    out = nc.dram_tensor("out", [T, DH], F32, kind="ExternalOutput").ap()

    NT = T // 512      # 8 token slices of 512
    NCT = D // 128     # 32 contraction tiles

    with tile.TileContext(nc) as tc:
        with tc.tile_pool(name="dram", bufs=1, space="DRAM") as dram:
            qTd = dram.tile([DH, T], BF16)
            kTd = dram.tile([DH, T], BF16)
            vd = dram.tile([T, DH], BF16)
            agin = {(b, h): dram.tile([128, S], BF16, name=f"agin{b}{h}")
                    for b in range(B) for h in range(HL)}
            agout = {(b, h): dram.tile([NC * 128, S], BF16, addr_space="Shared",
                                       name=f"agout{b}{h}")
                     for b in range(B) for h in range(HL)}

            # Outer SBUF pool spanning all phases (opened first so the
            # phase-1 pools below release their space LIFO for o_proj).
            with tc.tile_pool(name="attnS", bufs=1) as attnS:
                ob_sb = attnS.tile([128, 128], BF16, name="ob_sb")
                nc.sync.dma_start(ob_sb[:], ones_b[:])

                # ---------------- phase 1: QKV projections + RoPE ----------
                with tc.tile_pool(name="wres", bufs=1) as wres, \
                     tc.tile_pool(name="xs", bufs=36) as xs, \
                     tc.tile_pool(name="rp", bufs=2) as rp, \
                     tc.tile_pool(name="p1P", bufs=1, space="PSUM") as p1P:
                    # interleave wq and x(t0) load emission so the first
                    # matmul group's inputs land first; wk/wv (whose groups
                    # start ~7us later) follow
                    wtiles = {}
                    cos0 = rp.tile([128, 512], BF16, tag="cos_t", name="cos0")
                    nc.sync.dma_start(cos0[:], cos2E[:, 0:512])
                    sin0 = rp.tile([128, 512], BF16, tag="sin_t", name="sin0")
                    nc.sync.dma_start(sin0[:], sin2E[:, 0:512])
                    xt0 = []
                    for c in range(NCT):
                        wt = wres.tile([128, DH], BF16, name=f"wq_{c}")
                        nc.sync.dma_start(wt[:], wqT[c * 128:(c + 1) * 128, :])
                        wtiles[("q", c)] = wt
                        xc = xs.tile([128, 512], BF16, tag="xt", name=f"x_0_{c}")
                        nc.sync.dma_start(xc[:], xT[c * 128:(c + 1) * 128, 0:512])
                        xt0.append(xc)
                    for wname, w_dr in (("k", wkT), ("v", wvT)):
                        for c in range(NCT):
                            wt = wres.tile([128, DH], BF16, name=f"w{wname}_{c}")
                            nc.sync.dma_start(wt[:], w_dr[c * 128:(c + 1) * 128, :])
                            wtiles[(wname, c)] = wt

                    for t in range(NT):
                        tok = t * 512
                        if t == 0:
                            cos_t, sin_t, xt = cos0, sin0, xt0
                        else:
                            cos_t = rp.tile([128, 512], BF16, tag="cos_t", name=f"cos{t}")
                            nc.sync.dma_start(cos_t[:], cos2E[:, tok:tok + 512])
                            sin_t = rp.tile([128, 512], BF16, tag="sin_t", name=f"sin{t}")
                            nc.sync.dma_start(sin_t[:], sin2E[:, tok:tok + 512])
                            xt = []
                            for c in range(NCT):
                                xc = xs.tile([128, 512], BF16, tag="xt", name=f"x_{t}_{c}")
                                nc.sync.dma_start(xc[:], xT[c * 128:(c + 1) * 128, tok:tok + 512])
                                xt.append(xc)
                        # interleave q/k (rope) and v psum groups so psum
                        # drains overlap the next group's matmuls
                        groups = [("q", 0), ("k", 0), ("v", 0), ("q", 1), ("k", 1),
                                  ("v", 1), ("q", 2), ("k", 2), ("v", 2), ("q", 3),
                                  ("k", 3), ("v", 3)]
                        for wname, i in groups:
                            if wname == "v":
                                tt = i
                                psv = p1P.tile([128, 512], F32, tag="psv", bufs=4,
                                               name=f"psv_{t}_{tt}")
                                for c in range(NCT):
                                    nc.tensor.matmul(
                                        psv[:], xt[c][:, tt * 128:(tt + 1) * 128],
                                        wtiles[("v", c)][:],
                                        start=(c == 0), stop=(c == NCT - 1))
                                vsb = rp.tile([128, 512], BF16, tag="vsb",
                                              name=f"vsb{t}{tt}")
                                nc.scalar.activation(
                                    vsb[:], psv[:], mybir.ActivationFunctionType.Copy)
                                nc.sync.dma_start(
                                    vd[tok + tt * 128: tok + (tt + 1) * 128, :], vsb[:])
                                continue
                            dst = qTd if wname == "q" else kTd
                            ps = p1P.tile([128, 512], F32, tag="ps", bufs=4,
                                          name=f"ps_{t}_{wname}{i}")
                            for c in range(NCT):
                                nc.tensor.matmul(
                                    ps[:], wtiles[(wname, c)][:, i * 128:(i + 1) * 128],
                                    xt[c][:], start=(c == 0), stop=(c == NCT - 1))
                            qsb = rp.tile([128, 512], BF16, tag="qsb", name=f"qsb{t}{wname}{i}")
                            nc.scalar.activation(
                                qsb[:], ps[:], mybir.ActivationFunctionType.Copy)
                            # swap 64-row halves (x0 <-> x1) via sbuf-sbuf DMA
                            qs2 = rp.tile([128, 512], BF16, tag="qs2", name=f"qs2{t}{wname}{i}")
                            nc.sync.dma_start(qs2[0:64, :], qsb[64:128, :])
                            nc.sync.dma_start(qs2[64:128, :], qsb[0:64, :])
                            qc = rp.tile([128, 512], BF16, tag="qc", name=f"qc{t}{wname}{i}")
                            nc.vector.tensor_tensor(qc[:], qsb[:], cos_t[:], mybir.AluOpType.mult)
                            qr = rp.tile([128, 512], BF16, tag="qr", name=f"qr{t}{wname}{i}")
                            nc.vector.tensor_tensor(qr[:], qs2[:], sin_t[:], mybir.AluOpType.mult)
                            qfin = rp.tile([128, 512], BF16, tag="qfin", name=f"qf{t}{wname}{i}")
                            nc.vector.tensor_tensor(qfin[:], qc[:], qr[:], mybir.AluOpType.add)
                            nc.sync.dma_start(dst[i * 128:(i + 1) * 128, tok:tok + 512], qfin[:])

                # ---------------- phase 2: attention + AllGathers ----------
                with tc.tile_pool(name="attnP", bufs=1, space="PSUM") as attnP:
                    def issue_head_loads(b, h):
                        qh = attnS.tile([128, S], BF16, tag="qh", bufs=2, name=f"qh{b}{h}")
                        kh = attnS.tile([128, S], BF16, tag="kh", bufs=2, name=f"kh{b}{h}")
                        vh = attnS.tile([128, 16 * 128], BF16, tag="vh", bufs=2, name=f"vh{b}{h}")
                        nc.sync.dma_start(qh[:], qTd[h * 128:(h + 1) * 128, b * S:(b + 1) * S])
                        nc.sync.dma_start(kh[:], kTd[h * 128:(h + 1) * 128, b * S:(b + 1) * S])
                        nc.sync.dma_start(
                            vh[:].rearrange("p (kt d) -> p kt d", kt=16),
                            vd.rearrange("(bb kt p) i -> bb p kt i", bb=B, p=128)[b, :, :, h * 128:(h + 1) * 128])
                        return qh, kh, vh

                    heads = [(b, h) for b in range(B) for h in range(HL)]
                    pend = {heads[0]: issue_head_loads(*heads[0])}

                    def fire_ag(bh):
                        nc.gpsimd.collective_compute(
                            "AllGather", mybir.AluOpType.bypass,
                            replica_groups=[list(range(NC))],
                            ins=[agin[bh].opt()],
                            outs=[agout[bh].opt()])

                    pending_fin = []

                    for idx, (b, h) in enumerate(heads):
                        if idx + 1 < len(heads):
                            pend[heads[idx + 1]] = issue_head_loads(*heads[idx + 1])
                        qh, kh, vh = pend.pop((b, h))
                        for jq in range(4):
                            nkt = 4 * (jq + 1)
                            acc = attnP.tile([128, 512], F32, tag="acc", bufs=2,
                                             name=f"acc{b}{h}{jq}")
                            kts = list(range(nkt))
                            chunks = [kts[i:i + CH] for i in range(0, nkt, CH)]
                            lines = []
                            prev = None  # (chunk_kts, ex_tile) awaiting PV

                            def emit_pv(ch_kts, exc):
                                for si, kt in enumerate(ch_kts):
                                    d = kt - 4 * jq
                                    coff = 128 * d if d >= 0 else 0
                                    nc.tensor.matmul(
                                        acc[:, coff:], vh[:, kt * 128:(kt + 1) * 128],
                                        exc[:, si * 512 + coff:(si + 1) * 512],
                                        start=(kt == 0), stop=(kt == nkt - 1))

                            for ci, ch_kts in enumerate(chunks):
                                pss = attnP.tile([128, CH * 512], F32, tag="pss", bufs=2,
                                                 name=f"pss{b}{h}{jq}{ci}")
                                for si, kt in enumerate(ch_kts):
                                    d = kt - 4 * jq
                                    coff = 128 * d if d >= 0 else 0
                                    nc.tensor.matmul(
                                        pss[:, si * 512 + coff:(si + 1) * 512],
                                        kh[:, kt * 128:(kt + 1) * 128],
                                        qh[:, jq * 512 + coff:(jq + 1) * 512],
                                        start=True, stop=True)
                                # previous jq's normalization overlaps this
                                # jq's first score chunk on the tensor queue
                                if ci == 0 and pending_fin:
                                    pending_fin.pop(0)()
                                w = len(ch_kts) * 512
                                exc = attnS.tile([128, CH * 512], BF16, tag="ex", bufs=4,
                                                 name=f"ex{b}{h}{jq}{ci}")
                                nc.scalar.activation(
                                    exc[:, 0:w], pss[:, 0:w],
                                    mybir.ActivationFunctionType.Exp, scale=SCALE)
                                # causal mask on gpsimd: REPLACE key>query
                                # lanes with 0.0 (select, not arithmetic, so
                                # Inf/NaN from the untouched psum region of
                                # column-trimmed diagonal blocks can't leak)
                                for si, kt in enumerate(ch_kts):
                                    d = kt - 4 * jq
                                    if d >= 0:
                                        sl = exc[:, si * 512:(si + 1) * 512]
                                        nc.gpsimd.affine_select(
                                            out=sl, in_=sl,
                                            compare_op=mybir.AluOpType.is_ge,
                                            fill=0.0, base=-128 * d,
                                            pattern=[[1, 512]],
                                            channel_multiplier=-1)
                                # collapse chunk to one bf16 line (vector)
                                if len(ch_kts) == 1:
                                    lines.append(exc[:, 0:512])
                                else:
                                    tl = attnS.tile([128, 512], BF16, tag="csl", bufs=2,
                                                    name=f"csl{b}{h}{jq}{ci}")
                                    nc.vector.tensor_tensor(
                                        tl[:], exc[:, 0:512], exc[:, 512:1024],
                                        mybir.AluOpType.add)
                                    for si in range(2, len(ch_kts)):
                                        nc.vector.tensor_tensor(
                                            tl[:], tl[:], exc[:, si * 512:(si + 1) * 512],
                                            mybir.AluOpType.add)
                                    lines.append(tl[:])
                                if prev is not None:
                                    emit_pv(*prev)
                                prev = (ch_kts, exc)
                            emit_pv(*prev)
                            # fold lines -> bf16 column-sum (f32 partials)
                            cur = lines[0]
                            for li in lines[1:-1]:
                                if cur is lines[0]:
                                    f = attnS.tile([128, 512], F32, tag="cs", bufs=2,
                                                   name=f"cs{b}{h}{jq}")
                                    nc.vector.tensor_tensor(f[:], cur, li,
                                                            mybir.AluOpType.add)
                                    cur = f[:]
                                else:
                                    nc.vector.tensor_tensor(cur, cur, li,
                                                            mybir.AluOpType.add)
                            csb = attnS.tile([128, 512], BF16, tag="csb", bufs=2,
                                             name=f"csb{b}{h}{jq}")
                            nc.vector.tensor_tensor(csb[:], cur, lines[-1],
                                                    mybir.AluOpType.add)

                            def make_fin(acc=acc, csb=csb, b=b, h=h, jq=jq):
                                def fin():
                                    # 128-partition collapse + broadcast of
                                    # the key-sums in one rank-128 matmul
                                    rb = attnP.tile([128, CH * 512], F32, tag="pss",
                                                    bufs=2, name=f"rb{b}{h}{jq}")
                                    nc.tensor.matmul(rb[:, 0:512], ob_sb[:], csb[:],
                                                     start=True, stop=True)
                                    rec = attnS.tile([128, 512], F32, tag="rec", bufs=2,
                                                     name=f"rec{b}{h}{jq}")
                                    nc.vector.reciprocal_approx_fast(rec[:], rb[:, 0:512])
                                    att = attnS.tile([128, 512], BF16, tag="att", bufs=3,
                                                     name=f"att{b}{h}{jq}")
                                    nc.vector.tensor_tensor(att[:], acc[:], rec[:],
                                                            mybir.AluOpType.mult)
                                    nc.sync.dma_start(
                                        agin[(b, h)][:, jq * 512:(jq + 1) * 512], att[:])
                                return fin
                            pending_fin.append(make_fin())
                        # flush the head's remaining finalize, then fire its
                        # AllGather so link traffic spreads under compute
                        while pending_fin:
                            pending_fin.pop(0)()
                        fire_ag((b, h))

                # ---------------- phase 3: o_proj ----------------
                with tc.tile_pool(name="ores", bufs=1) as ores, \
                     tc.tile_pool(name="och", bufs=5) as och, \
                     tc.tile_pool(name="oo", bufs=4) as oo, \
                     tc.tile_pool(name="opP", bufs=1, space="PSUM") as opP:
                    wo_sb = ores.tile([128, NCT * DH], BF16, name="wo_sb")
                    nc.sync.dma_start(
                        wo_sb[:].rearrange("p (c i) -> p c i", c=NCT),
                        woT.rearrange("(c p) i -> p c i", p=128))
                    for t in range(T // 128):
                        bb = 0 if t < 16 else 1
                        tl = t % 16
                        ch = och.tile([128, NCT * 128], BF16, tag="ch", name=f"ch{t}")
                        # chunk c = r*4 + hh of the global head dim: gather the
                        # four per-head AllGather outputs side by side
                        chv = ch[:].rearrange("p (r hh u) -> p r hh u", r=NC, hh=HL)
                        for hh in range(HL):
                            nc.sync.dma_start(
                                chv[:, :, hh, :],
                                agout[(bb, hh)].rearrange("(r p) t -> p r t", p=128)[:, :, tl * 128:(tl + 1) * 128])
                        pso = opP.tile([128, 512], F32, tag="pso", bufs=3, name=f"pso{t}")
                        for i in range(NCT):
                            nc.tensor.matmul(pso[:], ch[:, i * 128:(i + 1) * 128],
                                             wo_sb[:, i * DH:(i + 1) * DH],
                                             start=(i == 0), stop=(i == NCT - 1))
                        ot = oo.tile([128, 512], F32, tag="ot", name=f"ot{t}")
                        nc.scalar.activation(
                            ot[:], pso[:], mybir.ActivationFunctionType.Copy)
                        nc.sync.dma_start(out[t * 128:(t + 1) * 128, :], ot[:])

    nc.compile()
    return nc


def _host_prep(x, freqs_cos, freqs_sin, mask, wq, wk, wv, wo):
    xT = np.ascontiguousarray(x.reshape(T, D).T).astype(bf16)
    cos = np.asarray(freqs_cos, np.float32).T   # [64, S]
    sin = np.asarray(freqs_sin, np.float32).T
    cos2 = np.concatenate([cos, cos], axis=0)           # [128, S]
    sin2 = np.concatenate([-sin, sin], axis=0)          # sign-folded
    cos2E = np.tile(cos2, (1, B)).astype(bf16)          # [128, T] b-major
    sin2E = np.tile(sin2, (1, B)).astype(bf16)
    # head-dim permutation: evens then odds within each 128-row head block
    perm = np.arange(D).reshape(H, HD // 2, 2).transpose(0, 2, 1).reshape(D)
    ones_b = np.ones((128, 128), bf16)
    shared = dict(xT=xT, cos2E=cos2E, sin2E=sin2E, ones_b=ones_b)
    wq_p = np.asarray(wq, np.float32)[perm, :]
    wk_p = np.asarray(wk, np.float32)[perm, :]
    in_maps = []
    for r in range(NC):
        sl = slice(r * DH, (r + 1) * DH)
        m = dict(shared)
        m["wqT"] = np.ascontiguousarray(wq_p[sl, :].T).astype(bf16)
        m["wkT"] = np.ascontiguousarray(wk_p[sl, :].T).astype(bf16)
        m["wvT"] = np.ascontiguousarray(np.asarray(wv, np.float32)[sl, :].T).astype(bf16)
        m["woT"] = np.ascontiguousarray(np.asarray(wo, np.float32)[sl, :].T).astype(bf16)
        in_maps.append(m)
    return in_maps


def kernel(x, freqs_cos, freqs_sin, mask, wq, wk, wv, wo, start_pos):
    global LAST_RESULT
    if "nc" not in _CACHE:
        _CACHE["nc"] = build()
    nc = _CACHE["nc"]
    in_maps = _host_prep(x, freqs_cos, freqs_sin, mask, wq, wk, wv, wo)
    res = run_bass_kernel_spmd(nc, in_maps, core_ids=list(range(NC)))
    LAST_RESULT = res
    parts = [res.results[r]["out"] for r in range(NC)]
    full = np.concatenate(parts, axis=1)      # [T, D]
    return np.ascontiguousarray(full.reshape(B, S, D)).astype(np.float32)


# revision 18
# speedup vs baseline: 1.0153x; 1.0153x over previous
"""Trainium2 8-core tensor-parallel attention kernel (Bass/Tile), v4.

Full inputs in, full output out. Sharding: tensor-parallel over heads
(4 heads per core), per-batch AllGather of attention outputs overlapped
with compute, each core computes a 512-wide output-column slice of the
o_proj; host concatenates.

v4 vs v2:
- Exp batched: scores for 2 key blocks accumulate in a 2-bank PSUM chunk
  and drain through ONE 1024-col activation (halves the count of ACT
  instructions, whose ~570ns fixed overhead made the scalar engine the
  attention bottleneck). The garbage psum region of column-trimmed
  diagonal blocks flows through exp but is never read (sums and PV
  matmuls stay column-trimmed).
- Normalization casts (recb/rbs) moved from the scalar engine (busy with
  exp) to the vector engine (light).
- Per-pair normalization is deferred one chunk into the next query block
  so its tensor-engine ops (broadcast matmuls) don't stall on the
  reciprocal chain.
- PSUM pools are phase-siblings (p1P -> attnP -> opP); phase-1 projection
  groups double-buffer 4-deep.
"""
import sys

for _p in ("/opt/trn_rl_repo",):
    if _p not in sys.path:
        sys.path.insert(0, _p)

import numpy as np
import ml_dtypes

import concourse.bass as bass
import concourse.mybir as mybir
import concourse.tile as tile
from concourse import bacc
from concourse.bass_utils import run_bass_kernel_spmd

B, S, D, H = 2, 2048, 4096, 32
HD = D // H          # 128 head dim
T = B * S            # 4096 tokens
NC = 8               # cores
HL = H // NC         # 4 heads per core
DH = HL * HD         # 512 dims per core
SCALE = 1.0 / float(np.sqrt(HD))
BF16 = mybir.dt.bfloat16
F32 = mybir.dt.float32
bf16 = ml_dtypes.bfloat16

_CACHE = {}
LAST_RESULT = None

CH = 2  # key blocks per psum score chunk (2 banks)


def build():
    nc = bacc.Bacc("TRN2", target_bir_lowering=False, debug=False, num_devices=NC)

    xT = nc.dram_tensor("xT", [D, T], BF16, kind="ExternalInput").ap()
    wqT = nc.dram_tensor("wqT", [D, DH], BF16, kind="ExternalInput").ap()
    wkT = nc.dram_tensor("wkT", [D, DH], BF16, kind="ExternalInput").ap()
    wvT = nc.dram_tensor("wvT", [D, DH], BF16, kind="ExternalInput").ap()
    woT = nc.dram_tensor("woT", [D, DH], BF16, kind="ExternalInput").ap()
    cos2E = nc.dram_tensor("cos2E", [HD, T], BF16, kind="ExternalInput").ap()
    sin2E = nc.dram_tensor("sin2E", [HD, T], BF16, kind="ExternalInput").ap()
    ones_k = nc.dram_tensor("ones_k", [128, 32], BF16, kind="ExternalInput").ap()
    ones_b = nc.dram_tensor("ones_b", [128, 128], BF16, kind="ExternalInput").ap()
    mask128 = nc.dram_tensor("mask128", [128, 128], F32, kind="ExternalInput").ap()
    out = nc.dram_tensor("out", [T, DH], F32, kind="ExternalOutput").ap()

    NT = T // 512      # 8 token slices of 512
    NCT = D // 128     # 32 contraction tiles

    with tile.TileContext(nc) as tc:
        with tc.tile_pool(name="dram", bufs=1, space="DRAM") as dram:
            qTd = dram.tile([DH, T], BF16)
            kTd = dram.tile([DH, T], BF16)
            vd = dram.tile([T, DH], BF16)
            agin = {(b, h): dram.tile([128, S], BF16, name=f"agin{b}{h}")
                    for b in range(B) for h in range(HL)}
            agout = {(b, h): dram.tile([NC * 128, S], BF16, addr_space="Shared",
                                       name=f"agout{b}{h}")
                     for b in range(B) for h in range(HL)}

            # Outer SBUF pool spanning all phases (opened first so the
            # phase-1 pools below release their space LIFO for o_proj).
            with tc.tile_pool(name="attnS", bufs=1) as attnS:
                ok_sb = attnS.tile([128, 32], BF16, name="ok_sb")
                nc.sync.dma_start(ok_sb[:], ones_k[:])
                ob_sb = attnS.tile([128, 128], BF16, name="ob_sb")
                nc.sync.dma_start(ob_sb[:], ones_b[:])
                mk_sb = attnS.tile([128, 128], F32, name="mk_sb")
                nc.sync.dma_start(mk_sb[:], mask128[:])

                # ---------------- phase 1: QKV projections + RoPE ----------
                with tc.tile_pool(name="wres", bufs=1) as wres, \
                     tc.tile_pool(name="xs", bufs=36) as xs, \
                     tc.tile_pool(name="rp", bufs=3) as rp, \
                     tc.tile_pool(name="p1P", bufs=1, space="PSUM") as p1P:
                    # interleave wq and x(t0) load emission so the first
                    # matmul group's inputs land first; wk/wv (whose groups
                    # start ~7us later) follow
                    wtiles = {}
                    cos0 = rp.tile([128, 512], BF16, tag="cos_t", name="cos0")
                    nc.sync.dma_start(cos0[:], cos2E[:, 0:512])
                    sin0 = rp.tile([128, 512], BF16, tag="sin_t", name="sin0")
                    nc.sync.dma_start(sin0[:], sin2E[:, 0:512])
                    xt0 = []
                    for c in range(NCT):
                        wt = wres.tile([128, DH], BF16, name=f"wq_{c}")
                        nc.sync.dma_start(wt[:], wqT[c * 128:(c + 1) * 128, :])
                        wtiles[("q", c)] = wt
                        xc = xs.tile([128, 512], BF16, tag="xt", name=f"x_0_{c}")
                        nc.sync.dma_start(xc[:], xT[c * 128:(c + 1) * 128, 0:512])
                        xt0.append(xc)
                    for wname, w_dr in (("k", wkT), ("v", wvT)):
                        for c in range(NCT):
                            wt = wres.tile([128, DH], BF16, name=f"w{wname}_{c}")
                            nc.sync.dma_start(wt[:], w_dr[c * 128:(c + 1) * 128, :])
                            wtiles[(wname, c)] = wt

                    for t in range(NT):
                        tok = t * 512
                        if t == 0:
                            cos_t, sin_t, xt = cos0, sin0, xt0
                        else:
                            cos_t = rp.tile([128, 512], BF16, tag="cos_t", name=f"cos{t}")
                            nc.sync.dma_start(cos_t[:], cos2E[:, tok:tok + 512])
                            sin_t = rp.tile([128, 512], BF16, tag="sin_t", name=f"sin{t}")
                            nc.sync.dma_start(sin_t[:], sin2E[:, tok:tok + 512])
                            xt = []
                            for c in range(NCT):
                                xc = xs.tile([128, 512], BF16, tag="xt", name=f"x_{t}_{c}")
                                nc.sync.dma_start(xc[:], xT[c * 128:(c + 1) * 128, tok:tok + 512])
                                xt.append(xc)
                        # interleave q/k (rope) and v psum groups so psum
                        # drains overlap the next group's matmuls
                        groups = [("q", 0), ("k", 0), ("v", 0), ("q", 1), ("k", 1),
                                  ("v", 1), ("q", 2), ("k", 2), ("v", 2), ("q", 3),
                                  ("k", 3), ("v", 3)]
                        for wname, i in groups:
                            if wname == "v":
                                tt = i
                                psv = p1P.tile([128, 512], F32, tag="psv", bufs=4,
                                               name=f"psv_{t}_{tt}")
                                for c in range(NCT):
                                    nc.tensor.matmul(
                                        psv[:], xt[c][:, tt * 128:(tt + 1) * 128],
                                        wtiles[("v", c)][:],
                                        start=(c == 0), stop=(c == NCT - 1))
                                vsb = rp.tile([128, 512], BF16, tag="vsb",
                                              name=f"vsb{t}{tt}")
                                nc.scalar.activation(
                                    vsb[:], psv[:], mybir.ActivationFunctionType.Copy)
                                nc.sync.dma_start(
                                    vd[tok + tt * 128: tok + (tt + 1) * 128, :], vsb[:])
                                continue
                            dst = qTd if wname == "q" else kTd
                            ps = p1P.tile([128, 512], F32, tag="ps", bufs=4,
                                          name=f"ps_{t}_{wname}{i}")
                            for c in range(NCT):
                                nc.tensor.matmul(
                                    ps[:], wtiles[(wname, c)][:, i * 128:(i + 1) * 128],
                                    xt[c][:], start=(c == 0), stop=(c == NCT - 1))
                            qsb = rp.tile([128, 512], BF16, tag="qsb", name=f"qsb{t}{wname}{i}")
                            nc.scalar.activation(
                                qsb[:], ps[:], mybir.ActivationFunctionType.Copy)
                            # swap 64-row halves (x0 <-> x1) via sbuf-sbuf DMA
                            qs2 = rp.tile([128, 512], BF16, tag="qs2", name=f"qs2{t}{wname}{i}")
                            nc.sync.dma_start(qs2[0:64, :], qsb[64:128, :])
                            nc.sync.dma_start(qs2[64:128, :], qsb[0:64, :])
                            qc = rp.tile([128, 512], BF16, tag="qc", name=f"qc{t}{wname}{i}")
                            nc.vector.tensor_tensor(qc[:], qsb[:], cos_t[:], mybir.AluOpType.mult)
                            qr = rp.tile([128, 512], BF16, tag="qr", name=f"qr{t}{wname}{i}")
                            nc.vector.tensor_tensor(qr[:], qs2[:], sin_t[:], mybir.AluOpType.mult)
                            qfin = rp.tile([128, 512], BF16, tag="qfin", name=f"qf{t}{wname}{i}")
                            nc.vector.tensor_tensor(qfin[:], qc[:], qr[:], mybir.AluOpType.add)
                            nc.sync.dma_start(dst[i * 128:(i + 1) * 128, tok:tok + 512], qfin[:])

                # ---------------- phase 2: attention + AllGathers ----------
                with tc.tile_pool(name="attnP", bufs=1, space="PSUM") as attnP:
                    def issue_head_loads(b, h):
                        qh = attnS.tile([128, S], BF16, tag="qh", bufs=2, name=f"qh{b}{h}")
                        kh = attnS.tile([128, S], BF16, tag="kh", bufs=2, name=f"kh{b}{h}")
                        vh = attnS.tile([128, 16 * 128], BF16, tag="vh", bufs=2, name=f"vh{b}{h}")
                        nc.sync.dma_start(qh[:], qTd[h * 128:(h + 1) * 128, b * S:(b + 1) * S])
                        nc.sync.dma_start(kh[:], kTd[h * 128:(h + 1) * 128, b * S:(b + 1) * S])
                        nc.sync.dma_start(
                            vh[:].rearrange("p (kt d) -> p kt d", kt=16),
                            vd.rearrange("(bb kt p) i -> bb p kt i", bb=B, p=128)[b, :, :, h * 128:(h + 1) * 128])
                        return qh, kh, vh

                    heads = [(b, h) for b in range(B) for h in range(HL)]
                    pend = {heads[0]: issue_head_loads(*heads[0])}

                    def fire_ag(bh):
                        nc.gpsimd.collective_compute(
                            "AllGather", mybir.AluOpType.bypass,
                            replica_groups=[list(range(NC))],
                            ins=[agin[bh].opt()],
                            outs=[agout[bh].opt()])

                    pending_fin = []

                    for idx, (b, h) in enumerate(heads):
                        if idx + 1 < len(heads):
                            pend[heads[idx + 1]] = issue_head_loads(*heads[idx + 1])
                        qh, kh, vh = pend.pop((b, h))
                        for pair in range(2):
                            sumsP = attnP.tile([128, 512], F32, tag="sums", bufs=1,
                                               name=f"sums{b}{h}{pair}")
                            accs = []
                            for l in range(2):
                                jq = 2 * pair + l
                                nkt = 4 * (jq + 1)
                                acc = attnP.tile([128, 512], F32, tag="acc", bufs=3,
                                                 name=f"acc{b}{h}{jq}")
                                accs.append(acc)
                                kts = list(range(nkt))
                                chunks = [kts[i:i + CH] for i in range(0, nkt, CH)]
                                prev = None  # (chunk_kts, ex_tile) awaiting PV

                                def emit_pv(ch_kts, exc, jq=jq, nkt=nkt, acc=acc,
                                            vh=vh, l=l, sumsP=sumsP):
                                    for si, kt in enumerate(ch_kts):
                                        d = kt - 4 * jq
                                        coff = 128 * d if d >= 0 else 0
                                        nc.tensor.matmul(
                                            acc[:, coff:], vh[:, kt * 128:(kt + 1) * 128],
                                            exc[:, si * 512 + coff:(si + 1) * 512],
                                            start=(kt == 0), stop=(kt == nkt - 1))
                                        nc.tensor.matmul(
                                            sumsP[32 * l:32 * l + 32, coff:],
                                            ok_sb[:], exc[:, si * 512 + coff:(si + 1) * 512],
                                            start=(kt == 0), stop=(kt == nkt - 1))

                                for ci, ch_kts in enumerate(chunks):
                                    pss = attnP.tile([128, CH * 512], F32, tag="pss",
                                                     bufs=2, name=f"pss{b}{h}{jq}{ci}")
                                    for si, kt in enumerate(ch_kts):
                                        d = kt - 4 * jq
                                        coff = 128 * d if d >= 0 else 0
                                        nc.tensor.matmul(
                                            pss[:, si * 512 + coff:(si + 1) * 512],
                                            kh[:, kt * 128:(kt + 1) * 128],
                                            qh[:, jq * 512 + coff:(jq + 1) * 512],
                                            start=True, stop=True)
                                        if d >= 0:
                                            # additive causal mask on the
                                            # diagonal 128x128 block
                                            nc.vector.tensor_tensor(
                                                pss[:, si * 512 + coff:si * 512 + coff + 128],
                                                pss[:, si * 512 + coff:si * 512 + coff + 128],
                                                mk_sb[:], mybir.AluOpType.add)
                                    # previous pair's normalization overlaps
                                    # this chunk on the tensor queue
                                    if pending_fin:
                                        pending_fin.pop(0)()
                                    w = len(ch_kts) * 512
                                    exc = attnS.tile([128, CH * 512], BF16, tag="ex",
                                                     bufs=4, name=f"ex{b}{h}{jq}{ci}")
                                    nc.scalar.activation(
                                        exc[:, 0:w], pss[:, 0:w],
                                        mybir.ActivationFunctionType.Exp, scale=SCALE)
                                    if prev is not None:
                                        emit_pv(*prev)
                                    prev = (ch_kts, exc)
                                emit_pv(*prev)

                            def make_fin(accs=accs, sumsP=sumsP, b=b, h=h, pair=pair):
                                def fin():
                                    recf = attnS.tile([64, 512], F32, tag="recf", bufs=2,
                                                      name=f"recf{b}{h}{pair}")
                                    nc.vector.reciprocal_approx_fast(recf[:], sumsP[0:64, :])
                                    recb = attnS.tile([64, 512], BF16, tag="recb", bufs=2,
                                                      name=f"recb{b}{h}{pair}")
                                    nc.vector.tensor_copy(recb[:], recf[:])
                                    for l in range(2):
                                        jq = 2 * pair + l
                                        rb = attnP.tile([128, CH * 512], F32, tag="pss",
                                                        bufs=2, name=f"rb{b}{h}{jq}")
                                        nc.tensor.matmul(rb[:, 0:512],
                                                         ob_sb[32 * l:32 * l + 1, :],
                                                         recb[32 * l:32 * l + 1, :],
                                                         start=True, stop=True)
                                        rbs = attnS.tile([128, 512], BF16, tag="rbs",
                                                         bufs=2, name=f"rbs{b}{h}{jq}")
                                        nc.vector.tensor_copy(rbs[:], rb[:, 0:512])
                                        att = attnS.tile([128, 512], BF16, tag="att",
                                                         bufs=3, name=f"att{b}{h}{jq}")
                                        nc.vector.tensor_tensor(att[:], accs[l][:], rbs[:],
                                                                mybir.AluOpType.mult)
                                        nc.sync.dma_start(
                                            agin[(b, h)][:, jq * 512:(jq + 1) * 512], att[:])
                                return fin
                            pending_fin.append(make_fin())
                        # flush the head's remaining finalize, then fire its
                        # AllGather so link traffic spreads under compute
                        while pending_fin:
                            pending_fin.pop(0)()
                        fire_ag((b, h))

                # ---------------- phase 3: o_proj ----------------
                with tc.tile_pool(name="ores", bufs=1) as ores, \
                     tc.tile_pool(name="och", bufs=5) as och, \
                     tc.tile_pool(name="oo", bufs=4) as oo, \
                     tc.tile_pool(name="opP", bufs=1, space="PSUM") as opP:
                    wo_sb = ores.tile([128, NCT * DH], BF16, name="wo_sb")
                    nc.sync.dma_start(
                        wo_sb[:].rearrange("p (c i) -> p c i", c=NCT),
                        woT.rearrange("(c p) i -> p c i", p=128))
                    for t in range(T // 128):
                        bb = 0 if t < 16 else 1
                        tl = t % 16
                        ch = och.tile([128, NCT * 128], BF16, tag="ch", name=f"ch{t}")
                        # chunk c = r*4 + hh of the global head dim: gather the
                        # four per-head AllGather outputs side by side
                        chv = ch[:].rearrange("p (r hh u) -> p r hh u", r=NC, hh=HL)
                        for hh in range(HL):
                            nc.sync.dma_start(
                                chv[:, :, hh, :],
                                agout[(bb, hh)].rearrange("(r p) t -> p r t", p=128)[:, :, tl * 128:(tl + 1) * 128])
                        pso = opP.tile([128, 512], F32, tag="pso", bufs=3, name=f"pso{t}")
                        for i in range(NCT):
                            nc.tensor.matmul(pso[:], ch[:, i * 128:(i + 1) * 128],
                                             wo_sb[:, i * DH:(i + 1) * DH],
                                             start=(i == 0), stop=(i == NCT - 1))
                        ot = oo.tile([128, 512], F32, tag="ot", name=f"ot{t}")
                        nc.scalar.activation(
                            ot[:], pso[:], mybir.ActivationFunctionType.Copy)
                        nc.sync.dma_start(out[t * 128:(t + 1) * 128, :], ot[:])

    nc.compile()
    return nc


def _host_prep(x, freqs_cos, freqs_sin, mask, wq, wk, wv, wo):
    xT = np.ascontiguousarray(x.reshape(T, D).T).astype(bf16)
    cos = np.asarray(freqs_cos, np.float32).T   # [64, S]
    sin = np.asarray(freqs_sin, np.float32).T
    cos2 = np.concatenate([cos, cos], axis=0)           # [128, S]
    sin2 = np.concatenate([-sin, sin], axis=0)          # sign-folded
    cos2E = np.tile(cos2, (1, B)).astype(bf16)          # [128, T] b-major
    sin2E = np.tile(sin2, (1, B)).astype(bf16)
    # head-dim permutation: evens then odds within each 128-row head block
    perm = np.arange(D).reshape(H, HD // 2, 2).transpose(0, 2, 1).reshape(D)
    ones_k = np.ones((128, 32), bf16)
    ones_b = np.ones((128, 128), bf16)
    # rows = keys, cols = queries: mask key>query = strictly lower triangle
    mask128 = np.tril(np.full((128, 128), -1e9, np.float32), k=-1)
    shared = dict(xT=xT, cos2E=cos2E, sin2E=sin2E, ones_k=ones_k, ones_b=ones_b,
                  mask128=mask128)
    wq_p = np.asarray(wq, np.float32)[perm, :]
    wk_p = np.asarray(wk, np.float32)[perm, :]
    in_maps = []
    for r in range(NC):
        sl = slice(r * DH, (r + 1) * DH)
        m = dict(shared)
        m["wqT"] = np.ascontiguousarray(wq_p[sl, :].T).astype(bf16)
        m["wkT"] = np.ascontiguousarray(wk_p[sl, :].T).astype(bf16)
        m["wvT"] = np.ascontiguousarray(np.asarray(wv, np.float32)[sl, :].T).astype(bf16)
        m["woT"] = np.ascontiguousarray(np.asarray(wo, np.float32)[sl, :].T).astype(bf16)
        in_maps.append(m)
    return in_maps


def kernel(x, freqs_cos, freqs_sin, mask, wq, wk, wv, wo, start_pos):
    global LAST_RESULT
    if "nc" not in _CACHE:
        _CACHE["nc"] = build()
    nc = _CACHE["nc"]
    in_maps = _host_prep(x, freqs_cos, freqs_sin, mask, wq, wk, wv, wo)
    res = run_bass_kernel_spmd(nc, in_maps, core_ids=list(range(NC)))
    LAST_RESULT = res
    parts = [res.results[r]["out"] for r in range(NC)]
    full = np.concatenate(parts, axis=1)      # [T, D]
    return np.ascontiguousarray(full.reshape(B, S, D)).astype(np.float32)


# revision 22
# speedup vs baseline: 1.0310x; 1.0155x over previous
"""Trainium2 8-core tensor-parallel attention kernel (Bass/Tile), v4.

Full inputs in, full output out. Sharding: tensor-parallel over heads
(4 heads per core), per-batch AllGather of attention outputs overlapped
with compute, each core computes a 512-wide output-column slice of the
o_proj; host concatenates.

v4 vs v2:
- Exp batched: scores for 2 key blocks accumulate in a 2-bank PSUM chunk
  and drain through ONE 1024-col activation (halves the count of ACT
  instructions, whose ~570ns fixed overhead made the scalar engine the
  attention bottleneck). The garbage psum region of column-trimmed
  diagonal blocks flows through exp but is never read (sums and PV
  matmuls stay column-trimmed).
- Normalization casts (recb/rbs) moved from the scalar engine (busy with
  exp) to the vector engine (light).
- Per-pair normalization is deferred one chunk into the next query block
  so its tensor-engine ops (broadcast matmuls) don't stall on the
  reciprocal chain.
- PSUM pools are phase-siblings (p1P -> attnP -> opP); phase-1 projection
  groups double-buffer 4-deep.
"""
import sys

for _p in ("/opt/trn_rl_repo",):
    if _p not in sys.path:
        sys.path.insert(0, _p)

import numpy as np
import ml_dtypes

import concourse.bass as bass
import concourse.mybir as mybir
import concourse.tile as tile
from concourse import bacc
from concourse.bass_utils import run_bass_kernel_spmd

B, S, D, H = 2, 2048, 4096, 32
HD = D // H          # 128 head dim
T = B * S            # 4096 tokens
NC = 8               # cores
HL = H // NC         # 4 heads per core
DH = HL * HD         # 512 dims per core
SCALE = 1.0 / float(np.sqrt(HD))
BF16 = mybir.dt.bfloat16
F32 = mybir.dt.float32
bf16 = ml_dtypes.bfloat16

_CACHE = {}
LAST_RESULT = None

CH = 2  # key blocks per psum score chunk (2 banks)


def build():
    nc = bacc.Bacc("TRN2", target_bir_lowering=False, debug=False, num_devices=NC)

    xT = nc.dram_tensor("xT", [D, T], BF16, kind="ExternalInput").ap()
    wqT = nc.dram_tensor("wqT", [D, DH], BF16, kind="ExternalInput").ap()
    wkT = nc.dram_tensor("wkT", [D, DH], BF16, kind="ExternalInput").ap()
    wvT = nc.dram_tensor("wvT", [D, DH], BF16, kind="ExternalInput").ap()
    woT = nc.dram_tensor("woT", [D, DH], BF16, kind="ExternalInput").ap()
    cos2E = nc.dram_tensor("cos2E", [HD, T], BF16, kind="ExternalInput").ap()
    sin2E = nc.dram_tensor("sin2E", [HD, T], BF16, kind="ExternalInput").ap()
    ones_k = nc.dram_tensor("ones_k", [128, 32], BF16, kind="ExternalInput").ap()
    ones_b = nc.dram_tensor("ones_b", [128, 128], BF16, kind="ExternalInput").ap()
    mask128 = nc.dram_tensor("mask128", [128, 128], F32, kind="ExternalInput").ap()
    out = nc.dram_tensor("out", [T, DH], F32, kind="ExternalOutput").ap()

    NT = T // 512      # 8 token slices of 512
    NCT = D // 128     # 32 contraction tiles

    with tile.TileContext(nc) as tc:
        with tc.tile_pool(name="dram", bufs=1, space="DRAM") as dram:
            qTd = dram.tile([DH, T], BF16)
            kTd = dram.tile([DH, T], BF16)
            vd = dram.tile([T, DH], BF16)
            agin = {(b, h): dram.tile([128, S], BF16, name=f"agin{b}{h}")
                    for b in range(B) for h in range(HL)}
            agout = {(b, h): dram.tile([NC * 128, S], BF16, addr_space="Shared",
                                       name=f"agout{b}{h}")
                     for b in range(B) for h in range(HL)}

            # Outer SBUF pool spanning all phases (opened first so the
            # phase-1 pools below release their space LIFO for o_proj).
            with tc.tile_pool(name="attnS", bufs=1) as attnS:
                ok_sb = attnS.tile([128, 32], BF16, name="ok_sb")
                nc.sync.dma_start(ok_sb[:], ones_k[:])
                ob_sb = attnS.tile([128, 128], BF16, name="ob_sb")
                nc.sync.dma_start(ob_sb[:], ones_b[:])
                mk_sb = attnS.tile([128, 128], F32, name="mk_sb")
                nc.sync.dma_start(mk_sb[:], mask128[:])

                # ---------------- phase 1: QKV projections + RoPE ----------
                with tc.tile_pool(name="wres", bufs=1) as wres, \
                     tc.tile_pool(name="xs", bufs=36) as xs, \
                     tc.tile_pool(name="rp", bufs=3) as rp, \
                     tc.tile_pool(name="p1P", bufs=1, space="PSUM") as p1P:
                    # interleave wq and x(t0) load emission so the first
                    # matmul group's inputs land first; wk/wv (whose groups
                    # start ~7us later) follow
                    wtiles = {}
                    cos0 = rp.tile([128, 512], BF16, tag="cos_t", name="cos0")
                    nc.sync.dma_start(cos0[:], cos2E[:, 0:512])
                    sin0 = rp.tile([128, 512], BF16, tag="sin_t", name="sin0")
                    nc.sync.dma_start(sin0[:], sin2E[:, 0:512])
                    xt0 = []
                    for c in range(NCT):
                        wt = wres.tile([128, DH], BF16, name=f"wq_{c}")
                        nc.sync.dma_start(wt[:], wqT[c * 128:(c + 1) * 128, :])
                        wtiles[("q", c)] = wt
                        xc = xs.tile([128, 512], BF16, tag="xt", name=f"x_0_{c}")
                        nc.sync.dma_start(xc[:], xT[c * 128:(c + 1) * 128, 0:512])
                        xt0.append(xc)
                    for wname, w_dr in (("k", wkT), ("v", wvT)):
                        for c in range(NCT):
                            wt = wres.tile([128, DH], BF16, name=f"w{wname}_{c}")
                            nc.sync.dma_start(wt[:], w_dr[c * 128:(c + 1) * 128, :])
                            wtiles[(wname, c)] = wt

                    for t in range(NT):
                        tok = t * 512
                        if t == 0:
                            cos_t, sin_t, xt = cos0, sin0, xt0
                        else:
                            cos_t = rp.tile([128, 512], BF16, tag="cos_t", name=f"cos{t}")
                            nc.sync.dma_start(cos_t[:], cos2E[:, tok:tok + 512])
                            sin_t = rp.tile([128, 512], BF16, tag="sin_t", name=f"sin{t}")
                            nc.sync.dma_start(sin_t[:], sin2E[:, tok:tok + 512])
                            xt = []
                            for c in range(NCT):
                                xc = xs.tile([128, 512], BF16, tag="xt", name=f"x_{t}_{c}")
                                nc.sync.dma_start(xc[:], xT[c * 128:(c + 1) * 128, tok:tok + 512])
                                xt.append(xc)
                        # interleave q/k (rope) and v psum groups so psum
                        # drains overlap the next group's matmuls
                        groups = [("q", 0), ("k", 0), ("v", 0), ("q", 1), ("k", 1),
                                  ("v", 1), ("q", 2), ("k", 2), ("v", 2), ("q", 3),
                                  ("k", 3), ("v", 3)]
                        for wname, i in groups:
                            if wname == "v":
                                tt = i
                                psv = p1P.tile([128, 512], F32, tag="psv", bufs=4,
                                               name=f"psv_{t}_{tt}")
                                for c in range(NCT):
                                    nc.tensor.matmul(
                                        psv[:], xt[c][:, tt * 128:(tt + 1) * 128],
                                        wtiles[("v", c)][:],
                                        start=(c == 0), stop=(c == NCT - 1))
                                vsb = rp.tile([128, 512], BF16, tag="vsb",
                                              name=f"vsb{t}{tt}")
                                nc.scalar.activation(
                                    vsb[:], psv[:], mybir.ActivationFunctionType.Copy)
                                nc.sync.dma_start(
                                    vd[tok + tt * 128: tok + (tt + 1) * 128, :], vsb[:])
                                continue
                            dst = qTd if wname == "q" else kTd
                            ps = p1P.tile([128, 512], F32, tag="ps", bufs=4,
                                          name=f"ps_{t}_{wname}{i}")
                            for c in range(NCT):
                                nc.tensor.matmul(
                                    ps[:], wtiles[(wname, c)][:, i * 128:(i + 1) * 128],
                                    xt[c][:], start=(c == 0), stop=(c == NCT - 1))
                            qsb = rp.tile([128, 512], BF16, tag="qsb", name=f"qsb{t}{wname}{i}")
                            nc.scalar.activation(
                                qsb[:], ps[:], mybir.ActivationFunctionType.Copy)
                            # swap 64-row halves (x0 <-> x1) via sbuf-sbuf DMA
                            qs2 = rp.tile([128, 512], BF16, tag="qs2", name=f"qs2{t}{wname}{i}")
                            nc.sync.dma_start(qs2[0:64, :], qsb[64:128, :])
                            nc.sync.dma_start(qs2[64:128, :], qsb[0:64, :])
                            qc = rp.tile([128, 512], BF16, tag="qc", name=f"qc{t}{wname}{i}")
                            nc.vector.tensor_tensor(qc[:], qsb[:], cos_t[:], mybir.AluOpType.mult)
                            qr = rp.tile([128, 512], BF16, tag="qr", name=f"qr{t}{wname}{i}")
                            nc.vector.tensor_tensor(qr[:], qs2[:], sin_t[:], mybir.AluOpType.mult)
                            qfin = rp.tile([128, 512], BF16, tag="qfin", name=f"qf{t}{wname}{i}")
                            nc.vector.tensor_tensor(qfin[:], qc[:], qr[:], mybir.AluOpType.add)
                            nc.sync.dma_start(dst[i * 128:(i + 1) * 128, tok:tok + 512], qfin[:])

                # ---------------- phase 2: attention + AllGathers ----------
                with tc.tile_pool(name="attnP", bufs=1, space="PSUM") as attnP:
                    def issue_head_loads(b, h):
                        qh = attnS.tile([128, S], BF16, tag="qh", bufs=2, name=f"qh{b}{h}")
                        kh = attnS.tile([128, S], BF16, tag="kh", bufs=2, name=f"kh{b}{h}")
                        vh = attnS.tile([128, 16 * 128], BF16, tag="vh", bufs=2, name=f"vh{b}{h}")
                        nc.sync.dma_start(qh[:], qTd[h * 128:(h + 1) * 128, b * S:(b + 1) * S])
                        nc.sync.dma_start(kh[:], kTd[h * 128:(h + 1) * 128, b * S:(b + 1) * S])
                        nc.sync.dma_start(
                            vh[:].rearrange("p (kt d) -> p kt d", kt=16),
                            vd.rearrange("(bb kt p) i -> bb p kt i", bb=B, p=128)[b, :, :, h * 128:(h + 1) * 128])
                        return qh, kh, vh

                    heads = [(b, h) for b in range(B) for h in range(HL)]
                    pend = {heads[0]: issue_head_loads(*heads[0])}

                    def fire_ag(bh):
                        nc.gpsimd.collective_compute(
                            "AllGather", mybir.AluOpType.bypass,
                            replica_groups=[list(range(NC))],
                            ins=[agin[bh].opt()],
                            outs=[agout[bh].opt()])

                    pending_fin = []  # (fin, ag_head_or_None)

                    def flush_one():
                        if pending_fin:
                            fin, ag = pending_fin.pop(0)
                            fin()
                            if ag is not None:
                                fire_ag(ag)

                    for idx, (b, h) in enumerate(heads):
                        if idx + 1 < len(heads):
                            pend[heads[idx + 1]] = issue_head_loads(*heads[idx + 1])
                        qh, kh, vh = pend.pop((b, h))
                        for pair in range(2):
                            sumsP = attnP.tile([128, 512], F32, tag="sums", bufs=1,
                                               name=f"sums{b}{h}{pair}")
                            accs = []
                            for l in range(2):
                                jq = 2 * pair + l
                                nkt = 4 * (jq + 1)
                                acc = attnP.tile([128, 512], F32, tag="acc", bufs=3,
                                                 name=f"acc{b}{h}{jq}")
                                accs.append(acc)
                                kts = list(range(nkt))
                                chunks = [kts[i:i + CH] for i in range(0, nkt, CH)]
                                fifo = []  # (chunk_kts, ex_tile) awaiting PV

                                def emit_pv(ch_kts, exc, jq=jq, nkt=nkt, acc=acc,
                                            vh=vh, l=l, sumsP=sumsP):
                                    for si, kt in enumerate(ch_kts):
                                        d = kt - 4 * jq
                                        coff = 128 * d if d >= 0 else 0
                                        nc.tensor.matmul(
                                            acc[:, coff:], vh[:, kt * 128:(kt + 1) * 128],
                                            exc[:, si * 512 + coff:(si + 1) * 512],
                                            start=(kt == 0), stop=(kt == nkt - 1))
                                        nc.tensor.matmul(
                                            sumsP[32 * l:32 * l + 32, coff:],
                                            ok_sb[:], exc[:, si * 512 + coff:(si + 1) * 512],
                                            start=(kt == 0), stop=(kt == nkt - 1))

                                for ci, ch_kts in enumerate(chunks):
                                    pss = attnP.tile([128, CH * 512], F32, tag="pss",
                                                     bufs=2, name=f"pss{b}{h}{jq}{ci}")
                                    for si, kt in enumerate(ch_kts):
                                        d = kt - 4 * jq
                                        coff = 128 * d if d >= 0 else 0
                                        nc.tensor.matmul(
                                            pss[:, si * 512 + coff:(si + 1) * 512],
                                            kh[:, kt * 128:(kt + 1) * 128],
                                            qh[:, jq * 512 + coff:(jq + 1) * 512],
                                            start=True, stop=True)
                                        if d >= 0:
                                            # additive causal mask on the
                                            # diagonal 128x128 block
                                            nc.vector.tensor_tensor(
                                                pss[:, si * 512 + coff:si * 512 + coff + 128],
                                                pss[:, si * 512 + coff:si * 512 + coff + 128],
                                                mk_sb[:], mybir.AluOpType.add)
                                    # previous pair's normalization (and the
                                    # previous head's AllGather) overlap this
                                    # chunk on the tensor queue
                                    flush_one()
                                    w = len(ch_kts) * 512
                                    d0 = ch_kts[0] - 4 * jq
                                    c0off = 128 * d0 if d0 >= 0 else 0
                                    exc = attnS.tile([128, CH * 512], BF16, tag="ex",
                                                     bufs=4, name=f"ex{b}{h}{jq}{ci}")
                                    nc.scalar.activation(
                                        exc[:, c0off:w], pss[:, c0off:w],
                                        mybir.ActivationFunctionType.Exp, scale=SCALE)
                                    # 2-chunk software pipeline: PV/sums run
                                    # two chunks behind scores so the exp
                                    # latency never stalls the tensor queue
                                    fifo.append((ch_kts, exc))
                                    if len(fifo) > 2:
                                        emit_pv(*fifo.pop(0))
                                for e in fifo:
                                    emit_pv(*e)

                            def make_fin(accs=accs, sumsP=sumsP, b=b, h=h, pair=pair):
                                def fin():
                                    recf = attnS.tile([64, 512], F32, tag="recf", bufs=2,
                                                      name=f"recf{b}{h}{pair}")
                                    nc.vector.reciprocal_approx_fast(recf[:], sumsP[0:64, :])
                                    recb = attnS.tile([64, 512], BF16, tag="recb", bufs=2,
                                                      name=f"recb{b}{h}{pair}")
                                    nc.vector.tensor_copy(recb[:], recf[:])
                                    for l in range(2):
                                        jq = 2 * pair + l
                                        rb = attnP.tile([128, CH * 512], F32, tag="pss",
                                                        bufs=2, name=f"rb{b}{h}{jq}")
                                        nc.tensor.matmul(rb[:, 0:512],
                                                         ob_sb[32 * l:32 * l + 1, :],
                                                         recb[32 * l:32 * l + 1, :],
                                                         start=True, stop=True)
                                        rbs = attnS.tile([128, 512], BF16, tag="rbs",
                                                         bufs=2, name=f"rbs{b}{h}{jq}")
                                        nc.vector.tensor_copy(rbs[:], rb[:, 0:512])
                                        att = attnS.tile([128, 512], BF16, tag="att",
                                                         bufs=3, name=f"att{b}{h}{jq}")
                                        nc.vector.tensor_tensor(att[:], accs[l][:], rbs[:],
                                                                mybir.AluOpType.mult)
                                        nc.sync.dma_start(
                                            agin[(b, h)][:, jq * 512:(jq + 1) * 512], att[:])
                                return fin
                            # the head's last pair carries its AllGather;
                            # both flush inside the NEXT head's first chunks
                            pending_fin.append(
                                (make_fin(), (b, h) if pair == 1 else None))
                    while pending_fin:
                        flush_one()

                # ---------------- phase 3: o_proj ----------------
                with tc.tile_pool(name="ores", bufs=1) as ores, \
                     tc.tile_pool(name="och", bufs=5) as och, \
                     tc.tile_pool(name="oo", bufs=4) as oo, \
                     tc.tile_pool(name="opP", bufs=1, space="PSUM") as opP:
                    wo_sb = ores.tile([128, NCT * DH], BF16, name="wo_sb")
                    nc.sync.dma_start(
                        wo_sb[:].rearrange("p (c i) -> p c i", c=NCT),
                        woT.rearrange("(c p) i -> p c i", p=128))
                    for t in range(T // 128):
                        bb = 0 if t < 16 else 1
                        tl = t % 16
                        ch = och.tile([128, NCT * 128], BF16, tag="ch", name=f"ch{t}")
                        # chunk c = r*4 + hh of the global head dim: gather the
                        # four per-head AllGather outputs side by side
                        chv = ch[:].rearrange("p (r hh u) -> p r hh u", r=NC, hh=HL)
                        for hh in range(HL):
                            nc.sync.dma_start(
                                chv[:, :, hh, :],
                                agout[(bb, hh)].rearrange("(r p) t -> p r t", p=128)[:, :, tl * 128:(tl + 1) * 128])
                        pso = opP.tile([128, 512], F32, tag="pso", bufs=3, name=f"pso{t}")
                        for i in range(NCT):
                            nc.tensor.matmul(pso[:], ch[:, i * 128:(i + 1) * 128],
                                             wo_sb[:, i * DH:(i + 1) * DH],
                                             start=(i == 0), stop=(i == NCT - 1))
                        ot = oo.tile([128, 512], F32, tag="ot", name=f"ot{t}")
                        nc.scalar.activation(
                            ot[:], pso[:], mybir.ActivationFunctionType.Copy)
                        nc.sync.dma_start(out[t * 128:(t + 1) * 128, :], ot[:])

    nc.compile()
    return nc


def _host_prep(x, freqs_cos, freqs_sin, mask, wq, wk, wv, wo):
    xT = np.ascontiguousarray(x.reshape(T, D).T).astype(bf16)
    cos = np.asarray(freqs_cos, np.float32).T   # [64, S]
    sin = np.asarray(freqs_sin, np.float32).T
    cos2 = np.concatenate([cos, cos], axis=0)           # [128, S]
    sin2 = np.concatenate([-sin, sin], axis=0)          # sign-folded
    cos2E = np.tile(cos2, (1, B)).astype(bf16)          # [128, T] b-major
    sin2E = np.tile(sin2, (1, B)).astype(bf16)
    # head-dim permutation: evens then odds within each 128-row head block
    perm = np.arange(D).reshape(H, HD // 2, 2).transpose(0, 2, 1).reshape(D)
    ones_k = np.ones((128, 32), bf16)
    ones_b = np.ones((128, 128), bf16)
    # rows = keys, cols = queries: mask key>query = strictly lower triangle
    mask128 = np.tril(np.full((128, 128), -1e9, np.float32), k=-1)
    shared = dict(xT=xT, cos2E=cos2E, sin2E=sin2E, ones_k=ones_k, ones_b=ones_b,
                  mask128=mask128)
    wq_p = np.asarray(wq, np.float32)[perm, :]
    wk_p = np.asarray(wk, np.float32)[perm, :]
    in_maps = []
    for r in range(NC):
        sl = slice(r * DH, (r + 1) * DH)
        m = dict(shared)
        m["wqT"] = np.ascontiguousarray(wq_p[sl, :].T).astype(bf16)
        m["wkT"] = np.ascontiguousarray(wk_p[sl, :].T).astype(bf16)
        m["wvT"] = np.ascontiguousarray(np.asarray(wv, np.float32)[sl, :].T).astype(bf16)
        m["woT"] = np.ascontiguousarray(np.asarray(wo, np.float32)[sl, :].T).astype(bf16)
        in_maps.append(m)
    return in_maps


def kernel(x, freqs_cos, freqs_sin, mask, wq, wk, wv, wo, start_pos):
    global LAST_RESULT
    if "nc" not in _CACHE:
        _CACHE["nc"] = build()
    nc = _CACHE["nc"]
    in_maps = _host_prep(x, freqs_cos, freqs_sin, mask, wq, wk, wv, wo)
    res = run_bass_kernel_spmd(nc, in_maps, core_ids=list(range(NC)))
    LAST_RESULT = res
    parts = [res.results[r]["out"] for r in range(NC)]
    full = np.concatenate(parts, axis=1)      # [T, D]
    return np.ascontiguousarray(full.reshape(B, S, D)).astype(np.float32)


# revision 23
# speedup vs baseline: 1.0312x; 1.0002x over previous
"""Trainium2 8-core tensor-parallel attention kernel (Bass/Tile), v4.

Full inputs in, full output out. Sharding: tensor-parallel over heads
(4 heads per core), per-batch AllGather of attention outputs overlapped
with compute, each core computes a 512-wide output-column slice of the
o_proj; host concatenates.

v4 vs v2:
- Exp batched: scores for 2 key blocks accumulate in a 2-bank PSUM chunk
  and drain through ONE 1024-col activation (halves the count of ACT
  instructions, whose ~570ns fixed overhead made the scalar engine the
  attention bottleneck). The garbage psum region of column-trimmed
  diagonal blocks flows through exp but is never read (sums and PV
  matmuls stay column-trimmed).
- Normalization casts (recb/rbs) moved from the scalar engine (busy with
  exp) to the vector engine (light).
- Per-pair normalization is deferred one chunk into the next query block
  so its tensor-engine ops (broadcast matmuls) don't stall on the
  reciprocal chain.
- PSUM pools are phase-siblings (p1P -> attnP -> opP); phase-1 projection
  groups double-buffer 4-deep.
"""
import sys

for _p in ("/opt/trn_rl_repo",):
    if _p not in sys.path:
        sys.path.insert(0, _p)

import numpy as np
import ml_dtypes

import concourse.bass as bass
import concourse.mybir as mybir
import concourse.tile as tile
from concourse import bacc
from concourse.bass_utils import run_bass_kernel_spmd

B, S, D, H = 2, 2048, 4096, 32
HD = D // H          # 128 head dim
T = B * S            # 4096 tokens
NC = 8               # cores
HL = H // NC         # 4 heads per core
DH = HL * HD         # 512 dims per core
SCALE = 1.0 / float(np.sqrt(HD))
BF16 = mybir.dt.bfloat16
F32 = mybir.dt.float32
bf16 = ml_dtypes.bfloat16

_CACHE = {}
LAST_RESULT = None

CH = 2  # key blocks per psum score chunk (2 banks)


def build():
    nc = bacc.Bacc("TRN2", target_bir_lowering=False, debug=False, num_devices=NC)

    xT = nc.dram_tensor("xT", [D, T], BF16, kind="ExternalInput").ap()
    wqT = nc.dram_tensor("wqT", [D, DH], BF16, kind="ExternalInput").ap()
    wkT = nc.dram_tensor("wkT", [D, DH], BF16, kind="ExternalInput").ap()
    wvT = nc.dram_tensor("wvT", [D, DH], BF16, kind="ExternalInput").ap()
    woT = nc.dram_tensor("woT", [D, DH], BF16, kind="ExternalInput").ap()
    cos2E = nc.dram_tensor("cos2E", [HD, T], BF16, kind="ExternalInput").ap()
    sin2E = nc.dram_tensor("sin2E", [HD, T], BF16, kind="ExternalInput").ap()
    ones_k = nc.dram_tensor("ones_k", [128, 32], BF16, kind="ExternalInput").ap()
    ones_b = nc.dram_tensor("ones_b", [128, 128], BF16, kind="ExternalInput").ap()
    mask128 = nc.dram_tensor("mask128", [128, 128], F32, kind="ExternalInput").ap()
    out = nc.dram_tensor("out", [T, DH], F32, kind="ExternalOutput").ap()

    NT = T // 512      # 8 token slices of 512
    NCT = D // 128     # 32 contraction tiles

    with tile.TileContext(nc) as tc:
        with tc.tile_pool(name="dram", bufs=1, space="DRAM") as dram:
            qTd = dram.tile([DH, T], BF16)
            kTd = dram.tile([DH, T], BF16)
            vd = dram.tile([T, DH], BF16)
            agin = {(b, h): dram.tile([128, S], BF16, name=f"agin{b}{h}")
                    for b in range(B) for h in range(HL)}
            agout = {(b, h): dram.tile([NC * 128, S], BF16, addr_space="Shared",
                                       name=f"agout{b}{h}")
                     for b in range(B) for h in range(HL)}

            # Outer SBUF pool spanning all phases (opened first so the
            # phase-1 pools below release their space LIFO for o_proj).
            with tc.tile_pool(name="attnS", bufs=1) as attnS:
                ok_sb = attnS.tile([128, 32], BF16, name="ok_sb")
                nc.sync.dma_start(ok_sb[:], ones_k[:])
                ob_sb = attnS.tile([128, 128], BF16, name="ob_sb")
                nc.sync.dma_start(ob_sb[:], ones_b[:])
                mk_sb = attnS.tile([128, 128], F32, name="mk_sb")
                nc.sync.dma_start(mk_sb[:], mask128[:])

                # ---------------- phase 1: QKV projections + RoPE ----------
                with tc.tile_pool(name="wres", bufs=1) as wres, \
                     tc.tile_pool(name="xs", bufs=36) as xs, \
                     tc.tile_pool(name="rp", bufs=3) as rp, \
                     tc.tile_pool(name="p1P", bufs=1, space="PSUM") as p1P:
                    # interleave wq and x(t0) load emission so the first
                    # matmul group's inputs land first; wk/wv (whose groups
                    # start ~7us later) follow
                    wtiles = {}
                    cos0 = rp.tile([128, 512], BF16, tag="cos_t", name="cos0")
                    nc.sync.dma_start(cos0[:], cos2E[:, 0:512])
                    sin0 = rp.tile([128, 512], BF16, tag="sin_t", name="sin0")
                    nc.sync.dma_start(sin0[:], sin2E[:, 0:512])
                    xt0 = []
                    for c in range(NCT):
                        wt = wres.tile([128, DH], BF16, name=f"wq_{c}")
                        nc.sync.dma_start(wt[:], wqT[c * 128:(c + 1) * 128, :])
                        wtiles[("q", c)] = wt
                        xc = xs.tile([128, 512], BF16, tag="xt", name=f"x_0_{c}")
                        nc.sync.dma_start(xc[:], xT[c * 128:(c + 1) * 128, 0:512])
                        xt0.append(xc)
                    for wname, w_dr in (("k", wkT), ("v", wvT)):
                        for c in range(NCT):
                            wt = wres.tile([128, DH], BF16, name=f"w{wname}_{c}")
                            nc.sync.dma_start(wt[:], w_dr[c * 128:(c + 1) * 128, :])
                            wtiles[(wname, c)] = wt

                    for t in range(NT):
                        tok = t * 512
                        if t == 0:
                            cos_t, sin_t, xt = cos0, sin0, xt0
                        else:
                            cos_t = rp.tile([128, 512], BF16, tag="cos_t", name=f"cos{t}")
                            nc.sync.dma_start(cos_t[:], cos2E[:, tok:tok + 512])
                            sin_t = rp.tile([128, 512], BF16, tag="sin_t", name=f"sin{t}")
                            nc.sync.dma_start(sin_t[:], sin2E[:, tok:tok + 512])
                            xt = []
                            for c in range(NCT):
                                xc = xs.tile([128, 512], BF16, tag="xt", name=f"x_{t}_{c}")
                                nc.sync.dma_start(xc[:], xT[c * 128:(c + 1) * 128, tok:tok + 512])
                                xt.append(xc)
                        # interleave q/k (rope) and v psum groups so psum
                        # drains overlap the next group's matmuls
                        groups = [("q", 0), ("k", 0), ("v", 0), ("q", 1), ("k", 1),
                                  ("v", 1), ("q", 2), ("k", 2), ("v", 2), ("q", 3),
                                  ("k", 3), ("v", 3)]
                        for wname, i in groups:
                            if wname == "v":
                                tt = i
                                psv = p1P.tile([128, 512], F32, tag="psv", bufs=4,
                                               name=f"psv_{t}_{tt}")
                                for c in range(NCT):
                                    nc.tensor.matmul(
                                        psv[:], xt[c][:, tt * 128:(tt + 1) * 128],
                                        wtiles[("v", c)][:],
                                        start=(c == 0), stop=(c == NCT - 1))
                                vsb = rp.tile([128, 512], BF16, tag="vsb",
                                              name=f"vsb{t}{tt}")
                                nc.scalar.activation(
                                    vsb[:], psv[:], mybir.ActivationFunctionType.Copy)
                                nc.sync.dma_start(
                                    vd[tok + tt * 128: tok + (tt + 1) * 128, :], vsb[:])
                                continue
                            dst = qTd if wname == "q" else kTd
                            ps = p1P.tile([128, 512], F32, tag="ps", bufs=4,
                                          name=f"ps_{t}_{wname}{i}")
                            for c in range(NCT):
                                nc.tensor.matmul(
                                    ps[:], wtiles[(wname, c)][:, i * 128:(i + 1) * 128],
                                    xt[c][:], start=(c == 0), stop=(c == NCT - 1))
                            qsb = rp.tile([128, 512], BF16, tag="qsb", name=f"qsb{t}{wname}{i}")
                            nc.scalar.activation(
                                qsb[:], ps[:], mybir.ActivationFunctionType.Copy)
                            # swap 64-row halves (x0 <-> x1) via sbuf-sbuf DMA
                            qs2 = rp.tile([128, 512], BF16, tag="qs2", name=f"qs2{t}{wname}{i}")
                            nc.sync.dma_start(qs2[0:64, :], qsb[64:128, :])
                            nc.sync.dma_start(qs2[64:128, :], qsb[0:64, :])
                            qc = rp.tile([128, 512], BF16, tag="qc", name=f"qc{t}{wname}{i}")
                            nc.vector.tensor_tensor(qc[:], qsb[:], cos_t[:], mybir.AluOpType.mult)
                            qr = rp.tile([128, 512], BF16, tag="qr", name=f"qr{t}{wname}{i}")
                            nc.vector.tensor_tensor(qr[:], qs2[:], sin_t[:], mybir.AluOpType.mult)
                            qfin = rp.tile([128, 512], BF16, tag="qfin", name=f"qf{t}{wname}{i}")
                            nc.vector.tensor_tensor(qfin[:], qc[:], qr[:], mybir.AluOpType.add)
                            nc.sync.dma_start(dst[i * 128:(i + 1) * 128, tok:tok + 512], qfin[:])

                # ---------------- phase 2: attention + AllGathers ----------
                with tc.tile_pool(name="attnP", bufs=1, space="PSUM") as attnP:
                    def issue_head_loads(b, h):
                        qh = attnS.tile([128, S], BF16, tag="qh", bufs=2, name=f"qh{b}{h}")
                        kh = attnS.tile([128, S], BF16, tag="kh", bufs=2, name=f"kh{b}{h}")
                        vh = attnS.tile([128, 16 * 128], BF16, tag="vh", bufs=2, name=f"vh{b}{h}")
                        nc.sync.dma_start(qh[:], qTd[h * 128:(h + 1) * 128, b * S:(b + 1) * S])
                        nc.sync.dma_start(kh[:], kTd[h * 128:(h + 1) * 128, b * S:(b + 1) * S])
                        nc.sync.dma_start(
                            vh[:].rearrange("p (kt d) -> p kt d", kt=16),
                            vd.rearrange("(bb kt p) i -> bb p kt i", bb=B, p=128)[b, :, :, h * 128:(h + 1) * 128])
                        return qh, kh, vh

                    heads = [(b, h) for b in range(B) for h in range(HL)]
                    pend = {heads[0]: issue_head_loads(*heads[0])}

                    def fire_ag(bh):
                        nc.gpsimd.collective_compute(
                            "AllGather", mybir.AluOpType.bypass,
                            replica_groups=[list(range(NC))],
                            ins=[agin[bh].opt()],
                            outs=[agout[bh].opt()])

                    pending_fin = []  # (fin, ag_head_or_None)

                    def flush_one():
                        if pending_fin:
                            fin, ag = pending_fin.pop(0)
                            fin()
                            if ag is not None:
                                fire_ag(ag)

                    for idx, (b, h) in enumerate(heads):
                        if idx + 1 < len(heads):
                            pend[heads[idx + 1]] = issue_head_loads(*heads[idx + 1])
                        qh, kh, vh = pend.pop((b, h))
                        for pair in range(2):
                            sumsP = attnP.tile([128, 512], F32, tag="sums", bufs=1,
                                               name=f"sums{b}{h}{pair}")
                            accs = []
                            for l in range(2):
                                jq = 2 * pair + l
                                nkt = 4 * (jq + 1)
                                acc = attnP.tile([128, 512], F32, tag="acc", bufs=3,
                                                 name=f"acc{b}{h}{jq}")
                                accs.append(acc)
                                kts = list(range(nkt))
                                chunks = [kts[i:i + CH] for i in range(0, nkt, CH)]
                                fifo = []  # (chunk_kts, ex_tile) awaiting PV

                                def emit_pv(ch_kts, exc, jq=jq, nkt=nkt, acc=acc,
                                            vh=vh, l=l, sumsP=sumsP):
                                    for si, kt in enumerate(ch_kts):
                                        d = kt - 4 * jq
                                        coff = 128 * d if d >= 0 else 0
                                        nc.tensor.matmul(
                                            acc[:, coff:], vh[:, kt * 128:(kt + 1) * 128],
                                            exc[:, si * 512 + coff:(si + 1) * 512],
                                            start=(kt == 0), stop=(kt == nkt - 1))
                                        nc.tensor.matmul(
                                            sumsP[32 * l:32 * l + 32, coff:],
                                            ok_sb[:], exc[:, si * 512 + coff:(si + 1) * 512],
                                            start=(kt == 0), stop=(kt == nkt - 1))

                                for ci, ch_kts in enumerate(chunks):
                                    pss = attnP.tile([128, CH * 512], F32, tag="pss",
                                                     bufs=2, name=f"pss{b}{h}{jq}{ci}")
                                    for si, kt in enumerate(ch_kts):
                                        d = kt - 4 * jq
                                        coff = 128 * d if d >= 0 else 0
                                        nc.tensor.matmul(
                                            pss[:, si * 512 + coff:(si + 1) * 512],
                                            kh[:, kt * 128:(kt + 1) * 128],
                                            qh[:, jq * 512 + coff:(jq + 1) * 512],
                                            start=True, stop=True)
                                        if d >= 0:
                                            # additive causal mask on the
                                            # diagonal 128x128 block
                                            nc.vector.tensor_tensor(
                                                pss[:, si * 512 + coff:si * 512 + coff + 128],
                                                pss[:, si * 512 + coff:si * 512 + coff + 128],
                                                mk_sb[:], mybir.AluOpType.add)
                                    # previous pair's normalization (and the
                                    # previous head's AllGather) overlap this
                                    # chunk on the tensor queue
                                    flush_one()
                                    w = len(ch_kts) * 512
                                    d0 = ch_kts[0] - 4 * jq
                                    c0off = 128 * d0 if d0 >= 0 else 0
                                    exc = attnS.tile([128, CH * 512], BF16, tag="ex",
                                                     bufs=4, name=f"ex{b}{h}{jq}{ci}")
                                    nc.scalar.activation(
                                        exc[:, c0off:w], pss[:, c0off:w],
                                        mybir.ActivationFunctionType.Exp, scale=SCALE)
                                    # 2-chunk software pipeline: PV/sums run
                                    # two chunks behind scores so the exp
                                    # latency never stalls the tensor queue
                                    fifo.append((ch_kts, exc))
                                    if len(fifo) > 2:
                                        emit_pv(*fifo.pop(0))
                                for e in fifo:
                                    emit_pv(*e)

                            # reciprocal chain is pure vector work: emit it
                            # NOW so it overlaps the next chunks; only the
                            # tensor-side broadcast + normalize is deferred
                            recf = attnS.tile([64, 512], F32, tag="recf", bufs=2,
                                              name=f"recf{b}{h}{pair}")
                            nc.vector.reciprocal_approx_fast(recf[:], sumsP[0:64, :])
                            recb = attnS.tile([64, 512], BF16, tag="recb", bufs=2,
                                              name=f"recb{b}{h}{pair}")
                            nc.vector.tensor_copy(recb[:], recf[:])

                            def make_fin(accs=accs, recb=recb, b=b, h=h, pair=pair):
                                def fin():
                                    for l in range(2):
                                        jq = 2 * pair + l
                                        rb = attnP.tile([128, CH * 512], F32, tag="pss",
                                                        bufs=2, name=f"rb{b}{h}{jq}")
                                        nc.tensor.matmul(rb[:, 0:512],
                                                         ob_sb[32 * l:32 * l + 1, :],
                                                         recb[32 * l:32 * l + 1, :],
                                                         start=True, stop=True)
                                        rbs = attnS.tile([128, 512], BF16, tag="rbs",
                                                         bufs=2, name=f"rbs{b}{h}{jq}")
                                        nc.vector.tensor_copy(rbs[:], rb[:, 0:512])
                                        att = attnS.tile([128, 512], BF16, tag="att",
                                                         bufs=3, name=f"att{b}{h}{jq}")
                                        nc.vector.tensor_tensor(att[:], accs[l][:], rbs[:],
                                                                mybir.AluOpType.mult)
                                        nc.sync.dma_start(
                                            agin[(b, h)][:, jq * 512:(jq + 1) * 512], att[:])
                                return fin
                            # the head's last pair carries its AllGather;
                            # both flush inside the NEXT head's first chunks
                            pending_fin.append(
                                (make_fin(), (b, h) if pair == 1 else None))
                    while pending_fin:
                        flush_one()

                # ---------------- phase 3: o_proj ----------------
                with tc.tile_pool(name="ores", bufs=1) as ores, \
                     tc.tile_pool(name="och", bufs=5) as och, \
                     tc.tile_pool(name="oo", bufs=4) as oo, \
                     tc.tile_pool(name="opP", bufs=1, space="PSUM") as opP:
                    wo_sb = ores.tile([128, NCT * DH], BF16, name="wo_sb")
                    nc.sync.dma_start(
                        wo_sb[:].rearrange("p (c i) -> p c i", c=NCT),
                        woT.rearrange("(c p) i -> p c i", p=128))
                    for t in range(T // 128):
                        bb = 0 if t < 16 else 1
                        tl = t % 16
                        ch = och.tile([128, NCT * 128], BF16, tag="ch", name=f"ch{t}")
                        # chunk c = r*4 + hh of the global head dim: gather the
                        # four per-head AllGather outputs side by side
                        chv = ch[:].rearrange("p (r hh u) -> p r hh u", r=NC, hh=HL)
                        for hh in range(HL):
                            nc.sync.dma_start(
                                chv[:, :, hh, :],
                                agout[(bb, hh)].rearrange("(r p) t -> p r t", p=128)[:, :, tl * 128:(tl + 1) * 128])
                        pso = opP.tile([128, 512], F32, tag="pso", bufs=3, name=f"pso{t}")
                        for i in range(NCT):
                            nc.tensor.matmul(pso[:], ch[:, i * 128:(i + 1) * 128],
                                             wo_sb[:, i * DH:(i + 1) * DH],
                                             start=(i == 0), stop=(i == NCT - 1))
                        ot = oo.tile([128, 512], F32, tag="ot", name=f"ot{t}")
                        nc.scalar.activation(
                            ot[:], pso[:], mybir.ActivationFunctionType.Copy)
                        nc.sync.dma_start(out[t * 128:(t + 1) * 128, :], ot[:])

    nc.compile()
    return nc


def _host_prep(x, freqs_cos, freqs_sin, mask, wq, wk, wv, wo):
    xT = np.ascontiguousarray(x.reshape(T, D).T).astype(bf16)
    cos = np.asarray(freqs_cos, np.float32).T   # [64, S]
    sin = np.asarray(freqs_sin, np.float32).T
    cos2 = np.concatenate([cos, cos], axis=0)           # [128, S]
    sin2 = np.concatenate([-sin, sin], axis=0)          # sign-folded
    cos2E = np.tile(cos2, (1, B)).astype(bf16)          # [128, T] b-major
    sin2E = np.tile(sin2, (1, B)).astype(bf16)
    # head-dim permutation: evens then odds within each 128-row head block
    perm = np.arange(D).reshape(H, HD // 2, 2).transpose(0, 2, 1).reshape(D)
    ones_k = np.ones((128, 32), bf16)
    ones_b = np.ones((128, 128), bf16)
    # rows = keys, cols = queries: mask key>query = strictly lower triangle
    mask128 = np.tril(np.full((128, 128), -1e9, np.float32), k=-1)
    shared = dict(xT=xT, cos2E=cos2E, sin2E=sin2E, ones_k=ones_k, ones_b=ones_b,
                  mask128=mask128)
    wq_p = np.asarray(wq, np.float32)[perm, :]
    wk_p = np.asarray(wk, np.float32)[perm, :]
    in_maps = []
    for r in range(NC):
        sl = slice(r * DH, (r + 1) * DH)
        m = dict(shared)
        m["wqT"] = np.ascontiguousarray(wq_p[sl, :].T).astype(bf16)
        m["wkT"] = np.ascontiguousarray(wk_p[sl, :].T).astype(bf16)
        m["wvT"] = np.ascontiguousarray(np.asarray(wv, np.float32)[sl, :].T).astype(bf16)
        m["woT"] = np.ascontiguousarray(np.asarray(wo, np.float32)[sl, :].T).astype(bf16)
        in_maps.append(m)
    return in_maps


def kernel(x, freqs_cos, freqs_sin, mask, wq, wk, wv, wo, start_pos):
    global LAST_RESULT
    if "nc" not in _CACHE:
        _CACHE["nc"] = build()
    nc = _CACHE["nc"]
    in_maps = _host_prep(x, freqs_cos, freqs_sin, mask, wq, wk, wv, wo)
    res = run_bass_kernel_spmd(nc, in_maps, core_ids=list(range(NC)))
    LAST_RESULT = res
    parts = [res.results[r]["out"] for r in range(NC)]
    full = np.concatenate(parts, axis=1)      # [T, D]
    return np.ascontiguousarray(full.reshape(B, S, D)).astype(np.float32)
